# revision 13
# baseline (speedup 1.0000x reference)
"""Trainium2 Bass kernel for nn_ConvAttention (sparse_attention).

Same algebra as the bf16 version (softmax is query-independent; the conv
weights fold the K-projection), but the score conv runs on the PE in fp8
DoubleRow mode: each matmul contracts TWO 128-deep k-tiles at 0.5
cycles/output-column, i.e. 4x the bf16 MAC rate.

Precision: e4m3 alone costs ~3e-2 relative error (gate is 2e-2), so the
conv is 3-term compensated exactly:
    W (x) x ~= Whi (x) xhi + Whi (x) xlo + Wlo (x) xhi
with Whi = fp8(W*SW), Wlo = fp8(W*SW - Whi), xhi = fp8(x*SX),
xlo = fp8(x*SX - xhi).  All terms share one PSUM accumulation at scale
SW*SX; the dropped Wlo(x)xlo term is ~1e-3 relative.  The cross terms
pair into single DoubleRow matmuls ((Whi,xlo)+(Wlo,xhi)), mains pair
across the p row-pair tiles, so a bank costs 23 DR matmuls vs 15 bf16
matmuls of twice the cycles: conv drops from ~5.9us to ~4.6us of PE.
The exp descale (1/(SW*SX)) folds into the ACT exp's scale input; the V
projection is also 3-term fp8 (scale folded into the V PSUM->SBUF copy).

Everything else (row-pair sharding, one-blob chunked DMA, dummy-matmul
p-state warmup, per-bank exp->mul->reduce chains, host-side divide) is
as in the bf16 kernel.
"""

import numpy as np

B, C, H, W, L = 2, 64, 16, 16, 32
NCORES = 8
NPAIR = 4
P = 2 * C
WCS0 = [(0, 4), (4, 6), (10, 6)]
WCS1 = [(0, 7), (13, 3), (7, 3), (10, 3)]
TAPORD = (2, 3, 4, 0, 1)
TAPIDX = {dx: i for i, dx in enumerate(TAPORD)}

SW = 32.0                   # conv-weight fp8 scale
SX = 8.0                    # x fp8 scale
SWV = 16.0                  # V-weight fp8 scale

# fp8 blob layout (1-byte elems per partition).  Taps store [hi|lo] pairs
# of 128x128 tiles; x pairs store [lo(512)|hi(512)] planes.
WT = {0: 0, 1: 2432, 2: 4736}
XP = {0: 1280, 1: 3712, 2: 6272, 3: 7296}
ZOFF = 2304                 # 128 zeros (DoubleRow padding partner)
WVOFF = 6016                # [wv_hi | wv_lo]
BLOB = 8320
CHUNKS = [(0, 2432), (2432, 2304), (4736, 2560), (7296, 1024)]

_PLAN = None


def _fp8():
    import ml_dtypes
    return ml_dtypes.float8_e4m3


def _np_bf16():
    import ml_dtypes
    return ml_dtypes.bfloat16


class _Plan:
    def __init__(self):
        import concourse.bacc as bacc
        import concourse.tile as tile
        from concourse import bass_types, mybir

        f32 = mybir.dt.float32
        bf16 = mybir.dt.bfloat16
        fp8 = mybir.dt.float8e4
        DR = mybir.MatmulPerfMode.DoubleRow
        nc = bacc.Bacc("TRN2", target_bir_lowering=False, debug=False,
                       num_devices=NCORES)

        d_d = [nc.dram_tensor(f"d{k}", [P, ln], fp8, kind="ExternalInput")
               for k, (st, ln) in enumerate(CHUNKS)]
        o_d = nc.dram_tensor("o", [P, 2, W], bf16, kind="ExternalOutput")
        o2_d = nc.dram_tensor("o2", [P, 2, W], bf16,
                              kind="ExternalOutput")

        with tile.TileContext(nc) as tc:
            with (
                tc.tile_pool(name="sb", bufs=1) as sb,
                tc.tile_pool(name="psum", bufs=1, space="PSUM") as psum,
            ):
                blob = sb.tile([P, BLOB], fp8, tag="blob")
                wdum = sb.tile([P, 128], bf16, tag="wdum")
                nc.gpsimd.memset(wdum[:], 0)

                for k, (st, ln) in enumerate(CHUNKS):
                    nc.sync.dma_start(out=blob[:, st:st + ln], in_=d_d[k][:])

                bt = blob[:]

                def wap(off, d):
                    # lhsT [K=128, 2, 128]: DoubleRow weight tile pair.
                    return bass_types.AP(bt.tensor, bt.offset + off,
                                         [[BLOB, P], [d, 2], [1, 128]])

                def xap(off, d, ncols):
                    # rhs [K=128, 2, ncols, L]: DoubleRow moving tile pair.
                    return bass_types.AP(bt.tensor, bt.offset + off,
                                         [[BLOB, P], [d, 2], [L, ncols],
                                          [1, L]])

                def sap(off, ncols):
                    # single-tile rhs [K, ncols, L] (sliver matmuls).
                    return bass_types.AP(bt.tensor, bt.offset + off,
                                         [[BLOB, P], [L, ncols], [1, L]])

                def wsap(off):
                    return bass_types.AP(bt.tensor, bt.offset + off,
                                         [[BLOB, P], [1, 128]])

                def wtoff(p, dx):
                    return WT[p] + 256 * TAPIDX[dx]

                # PSUM tiles are bank-granular (8 banks); rb1's last
                # bank reuses rb0-bank0's bank (their live ranges are
                # disjoint: the rb0-b0 exp reads it ~2.5us before rb1's
                # last conv bank starts accumulating).
                scores = {(rb, wci): psum.tile([P, n, L], f32,
                                               tag=f"s{rb}{wci}",
                                               name=f"s{rb}{wci}")
                          for rb, wcs in ((0, WCS0), (1, WCS1[:-1]))
                          for wci, (ws, n) in enumerate(wcs)}
                lwci = len(WCS1) - 1
                scores[(1, lwci)] = scores[(0, 0)][:, 0:WCS1[lwci][1], :]
                vps = [psum.tile([P, W, L], f32, tag=f"vp{t}", name=f"vp{t}")
                       for t in range(2)]
                vsb = [sb.tile([P, W, L], bf16, tag=f"vs{t}", name=f"vs{t}")
                       for t in range(2)]
                osum = sb.tile([P, 2, W], bf16, tag="os0", name="os0")
                osum1 = sb.tile([P, 2, W], bf16, tag="os1", name="os1")

                NWARM = 40
                for k in range(NWARM):
                    wid = 4 if k < 20 else 1
                    nc.tensor.matmul(
                        vps[1][:, 0:wid, 0:(L if wid == 4 else 1)],
                        lhsT=wdum[:], rhs=wdum[:, 0:(128 if wid == 4 else 1)],
                        start=(k == 0), stop=(k == NWARM - 1))

                def clip(ws, n, dx):
                    return max(ws, 2 - dx), min(ws + n, W + 2 - dx)

                def drmm(rb, wci, ws, woff, wd, xoff, xd, a, b,
                         start=False, stop=False):
                    nc.tensor.matmul(
                        scores[(rb, wci)][:, a - ws:b - ws, :]
                        if hasattr(scores[(rb, wci)], 'tensor')
                        else scores[(rb, wci)][:, a - ws:b - ws, :],
                        lhsT=wap(woff, wd),
                        rhs=xap(xoff + a * L, xd, b - a),
                        start=start, stop=stop, perf_mode=DR)

                def smm(rb, wci, ws, woff, xoff, a, b, stop=False):
                    nc.tensor.matmul(
                        scores[(rb, wci)][:, a - ws:b - ws, :],
                        lhsT=wsap(woff),
                        rhs=sap(xoff + a * L, b - a),
                        start=False, stop=stop)

                # Per (bank, p) emission so the schedule can follow the DMA
                # chunk stream.  p pass 0/1/2: cross terms for that p; the
                # p1 pass also does the (p0,p1) main pair; the p2 pass does
                # the p2 mains (zero-padded dx2 + cross-dx pairs + slivers).
                def conv_pass(rb, wci, ws, n, p):
                    xb = [XP[rb + q] for q in range(3)]
                    first = True
                    for dx in TAPORD:
                        a, b = clip(ws, n, dx)
                        if b <= a:
                            continue
                        sh = (dx - 2) * L
                        if p < 2:
                            # cross: Whi_p (x) xlo_p + Wlo_p (x) xhi_p
                            drmm(rb, wci, ws, wtoff(p, dx), 128,
                                 xb[p] + sh, 512, a, b,
                                 start=(p == 0 and first))
                            first = False
                            if p == 1:
                                # mains (p0, p1) on the hi planes
                                drmm(rb, wci, ws, wtoff(0, dx),
                                     wtoff(1, dx) - wtoff(0, dx),
                                     xb[0] + 512 + sh, xb[1] - xb[0], a, b)
                        else:
                            drmm(rb, wci, ws, wtoff(2, dx), 128,
                                 xb[2] + sh, 512, a, b)
                    if p == 2:
                        # p2 mains.  dx=2 (full range) pairs with the zero
                        # tile; (3,4) and (0,1) pair cross-dx on their range
                        # intersections, with sliver singles for the rest.
                        a, b = ws, ws + n
                        drmm(rb, wci, ws, ZOFF, wtoff(2, 2) - ZOFF,
                             xb[2], 512, a, b)
                        slv = []
                        for dxa, dxb in ((3, 4), (0, 1)):
                            aa, ba = clip(ws, n, dxa)
                            ab, bb = clip(ws, n, dxb)
                            ia, ib = max(aa, ab), min(ba, bb)
                            if ib > ia:
                                drmm(rb, wci, ws, wtoff(2, dxa),
                                     256 * (TAPIDX[dxb] - TAPIDX[dxa]),
                                     xb[2] + 512 + (dxa - 2) * L,
                                     (dxb - dxa) * L, ia, ib)
                            for dx, (c, d) in ((dxa, (aa, ba)),
                                               (dxb, (ab, bb))):
                                for sa, sb_ in ((c, min(d, ia)),
                                                (max(c, ib), d)):
                                    if sb_ > sa:
                                        slv.append((dx, sa, sb_))
                        for k, (dx, sa, sb_) in enumerate(slv):
                            smm(rb, wci, ws, wtoff(2, dx),
                                xb[2] + 512 + (dx - 2) * L, sa, sb_,
                                stop=(k == len(slv) - 1))
                        if not slv:
                            # close the accumulation group with a 1-col
                            # zero-weight single matmul (adds 0).
                            smm(rb, wci, ws, ZOFF, xb[2] + 512, ws, ws + 1,
                                stop=True)

                def conv_group(rb, p, wcs):
                    for wci, (ws, n) in enumerate(wcs):
                        conv_pass(rb, wci, ws, n, p)

                def conv_bank(rb, wci, ws, n):
                    for p in range(3):
                        conv_pass(rb, wci, ws, n, p)

                def vproj(rb):
                    # vps[rb] = wv (x) x_{rb+1}, 3-term fp8.
                    xo = XP[rb + 1]
                    nc.tensor.matmul(
                        vps[rb][:], lhsT=wap(WVOFF, 128),
                        rhs=xap(xo, 512, W), start=True, stop=False,
                        perf_mode=DR)
                    nc.tensor.matmul(
                        vps[rb][:], lhsT=wap(ZOFF, WVOFF - ZOFF),
                        rhs=xap(xo, 512, W), start=False, stop=True,
                        perf_mode=DR)

                def chain(rb, wci, ws, n, mule=None):
                    mule = mule or nc.vector
                    ee = sb.tile([P, 2, n, L], bf16, tag=f"e{rb}{wci}",
                                 name=f"e{rb}{wci}")
                    nc.scalar.activation(
                        ee[:, 0], scores[(rb, wci)][:],
                        func=mybir.ActivationFunctionType.Exp,
                        scale=1.0 / (SW * SX))
                    mule.tensor_mul(ee[:, 1], ee[:, 0],
                                    vsb[rb][:, ws:ws + n, :])
                    out_ap = (osum if rb == 0
                              else osum1)[:, :, ws:ws + n]
                    with nc.allow_low_precision(
                            reason="32-term bf16 sums; rel tol 2e-2"):
                        nc.vector.tensor_reduce(
                            out=out_ap, in_=ee[:],
                            axis=mybir.AxisListType.X,
                            op=mybir.AluOpType.add)

                conv_group(0, 0, WCS0)
                conv_group(0, 1, WCS0)
                vproj(0)
                nc.scalar.activation(vsb[0][:], vps[0][:],
                                     func=mybir.ActivationFunctionType.Copy,
                                     scale=1.0 / (SWV * SX))
                # bank-major p2 passes so bank0's scores (and its softmax
                # chain, which seeds the serial DVE tail) finish as soon as
                # chunk 3 lands.
                for wci, (ws, n) in enumerate(WCS0):
                    conv_pass(0, wci, ws, n, 2)
                    chain(0, wci, ws, n)
                    if wci == 0:
                        vproj(1)
                nc.scalar.activation(vsb[1][:], vps[1][:],
                                     func=mybir.ActivationFunctionType.Copy,
                                     scale=1.0 / (SWV * SX))
                nc.sync.dma_start(out=o_d[:], in_=osum[:])
                for wci, (ws, n) in enumerate(WCS1):
                    conv_bank(1, wci, ws, n)
                    chain(1, wci, ws, n)
                nc.sync.dma_start(
                    out=o2_d[:],
                    in_=osum1[:])

        nc.compile()
        self.nc = nc


def _get_plan():
    global _PLAN
    if _PLAN is None:
        _PLAN = _Plan()
    return _PLAN


def _prep_in_maps(x, W1, W2):
    fp8 = _fp8()

    W1k = W1[C:2 * C, :, 0, 0].astype(np.float64)
    W2eff = np.einsum("okyx,kc->ocyx", W2.astype(np.float64),
                      W1k).astype(np.float32)
    W1v = W1[2 * C:3 * C, :, 0, 0].astype(np.float32)

    wtiles = np.zeros((3, 5, P, P), np.float32)
    for p in range(3):
        for dx in range(5):
            for s in range(2):
                for rh in range(2):
                    dyi = 2 * p + s - rh
                    if 0 <= dyi <= 4:
                        wtiles[p, dx,
                               64 * s:64 * s + 64,
                               64 * rh:64 * rh + 64] = W2eff[:, :, dyi, dx].T
    wv = np.zeros((P, P), np.float32)
    wv[:C, :C] = W1v.T
    wv[C:, C:] = W1v.T

    def hilo(a, scale):
        hi = (a * scale).astype(fp8)
        lo = (a * scale - hi.astype(np.float32)).astype(fp8)
        return hi, lo

    whi, wlo = hilo(wtiles, SW)          # [3,5,P,P]
    wvhi, wvlo = hilo(wv, SWV)           # [P,P]

    xp = np.zeros((B, C, H + 4, W, L), np.float32)
    xp[:, :, 2:2 + H] = x
    in_maps = []
    for m in range(NCORES):
        bm, q = m // 4, m % 4
        rows = xp[bm, :, 4 * q:4 * q + 8]
        tiles = rows.reshape(C, NPAIR, 2, W, L).transpose(2, 0, 1, 3, 4)
        tiles = tiles.reshape(P, NPAIR, W * L)
        thi, tlo = hilo(tiles, SX)

        blob = np.zeros((P, BLOB), np.float32)
        for p in range(3):
            for dx in range(5):
                off = WT[p] + 256 * TAPIDX[dx]
                blob[:, off:off + 128] = whi[p, dx]
                blob[:, off + 128:off + 256] = wlo[p, dx]
        blob[:, WVOFF:WVOFF + 128] = wvhi
        blob[:, WVOFF + 128:WVOFF + 256] = wvlo
        bq = blob.astype(fp8)
        for t in range(NPAIR):
            bq[:, XP[t]:XP[t] + W * L] = tlo[:, t]
            bq[:, XP[t] + W * L:XP[t] + 2 * W * L] = thi[:, t]
        im = {f"d{k}": np.ascontiguousarray(bq[:, st:st + ln])
              for k, (st, ln) in enumerate(CHUNKS)}
        in_maps.append(im)
    return in_maps


def kernel(x, W1, b1, W2, b2):
    from concourse.bass_utils import run_bass_kernel_spmd

    x = np.asarray(x, dtype=np.float32)
    W1 = np.asarray(W1, dtype=np.float32)
    b1 = np.asarray(b1, dtype=np.float32)
    W2 = np.asarray(W2, dtype=np.float32)

    plan = _get_plan()
    in_maps = _prep_in_maps(x, W1, W2)
    res = run_bass_kernel_spmd(plan.nc, in_maps, core_ids=list(range(NCORES)))

    b1v = b1[2 * C:3 * C].astype(np.float32)
    out = np.empty((B, C, H, W, L), np.float32)
    for m in range(NCORES):
        bm, q = m // 4, m % 4
        o0 = np.asarray(res.results[m]["o"], dtype=np.float32)
        o1 = np.asarray(res.results[m]["o2"],
                        dtype=np.float32).reshape(P, 2, W)
        o = np.stack([o0.reshape(P, 2, W), o1], axis=1)
        o = o.reshape(2, C, 2, 2, W)
        val = o[:, :, :, 1] / o[:, :, :, 0]
        val = val + b1v[None, :, None, None]
        val = val.transpose(1, 2, 0, 3).reshape(C, 4, W)
        out[bm, :, 4 * q:4 * q + 4, :, :] = val[..., None]
    return out
